# revision 27
# baseline (speedup 1.0000x reference)
"""Trainium2 Bass kernel for nn_Decoder: 2-layer GRU decoder + LayerNorm + ELU + vocab head.

Contract: kernel(**inputs) takes the FULL unsharded inputs (as produced by the
reference setup_inputs) and returns the FULL (512, 64, 10000) float32 logits.
Internally: data-parallel shard of batch B=512 across 8 NeuronCores; all
weights replicated. Self-contained (shapes hardcoded).

Layout notes (per core, BS=64 batch rows):
- Both GRU layers run interleaved, one slot apart: slot s computes layer-0
  step s and layer-1 step s-1. State tile hh [64, 2, 256] = (b, layer, h)
  with layer 1 in slot 0. GRU matmuls use float32r (full PE rate).
- Output stores go through the gpsimd SWDGE queue: HWDGE (sync/scalar)
  SBUF->DRAM stores serialize onto 2 SDMA engines (~26 GB/s/queue), while
  SWDGE fans descriptors across all 16 engines (~358 GB/s HBM-bound).
  Each store is [128, 5000] f32 = 2.56 MB with 20 KB/partition descriptors.
- Head matmuls in bf16 (yT stationary, W_out.T moving): 2 cols/cycle on the
  PE + half the weight-load traffic. PSUM accumulates f32, logits exact
  enough (~1e-3 rel).
- ACT stays on the exp_and_others table set the whole kernel: tanh for GRU
  gates (sigmoid(x) = 0.5 + 0.5*tanh(x/2)), exp for ELU
  (elu(x) = max(x,0) + exp(min(x,0)) - 1). LN rsqrt = bit-trick + Newton
  (on GpSimd).
"""
import os
import sys

for _p in ("/opt/trn_rl_repo", "/root/.axon_site/_ro/trn_rl_repo"):
    if os.path.isdir(_p) and _p not in sys.path:
        sys.path.append(_p)

import numpy as np
import ml_dtypes
import concourse.bacc as bacc
import concourse.mybir as mybir
import concourse.tile as tile
from concourse.bass_utils import run_bass_kernel_spmd

F32 = mybir.dt.float32
F32R = mybir.dt.float32r
BF16 = mybir.dt.bfloat16
I32 = mybir.dt.int32
AF = mybir.ActivationFunctionType
ALU = mybir.AluOpType

B, Z, H, T, P = 512, 64, 256, 64, 10000
NCORES = 8
BS = B // NCORES
LN_EPS = 1e-5
NCH = 500                 # head N-chunk (PSUM bank = 512 f32; 20*500 = P)
HALF = P // 2             # store slice cols -> 20KB descriptors
RSQRT_NEWTON = 3
G = 8                     # y-path batch (steps)

last_exec_ns = None
last_results = None


def _np(x):
    return np.ascontiguousarray(np.asarray(x, dtype=np.float32))


def _build(flags):
    nc = bacc.Bacc("TRN2", target_bir_lowering=False)

    zT_d = nc.dram_tensor("zT", (Z, BS), F32R, kind="ExternalInput")
    winitT_d = nc.dram_tensor("winitT", (Z, H), F32R, kind="ExternalInput")
    whh0_d = nc.dram_tensor("whh0T", (2, 128, 3 * H), F32R, kind="ExternalInput")
    whh1_d = nc.dram_tensor("whh1T", (2, 128, 3 * H), F32R, kind="ExternalInput")
    wih1_d = nc.dram_tensor("wih1T", (2, 128, 3 * H), F32R, kind="ExternalInput")
    wout_d = nc.dram_tensor("woutT", (2, 128, P), BF16, kind="ExternalInput")
    ident_d = nc.dram_tensor("ident", (128, 128), F32, kind="ExternalInput")
    c0rz_d = nc.dram_tensor("c0rz", (1, 2 * H), F32R, kind="ExternalInput")
    c0n_d = nc.dram_tensor("c0n", (1, H), F32, kind="ExternalInput")
    if flags["binit"]:
        binit_d = nc.dram_tensor("binit", (1, H), F32, kind="ExternalInput")
    if flags["c1rz"]:
        c1rz_d = nc.dram_tensor("c1rz", (1, 2 * H), F32R, kind="ExternalInput")
    if flags["bhh0n"]:
        bhh0n_d = nc.dram_tensor("bhh0n", (1, H), F32R, kind="ExternalInput")
    if flags["bhh1n"]:
        bhh1n_d = nc.dram_tensor("bhh1n", (1, H), F32R, kind="ExternalInput")
    if flags["bih1n"]:
        bih1n_d = nc.dram_tensor("bih1n", (1, H), F32R, kind="ExternalInput")
    if flags["lng"]:
        lng_d = nc.dram_tensor("lng", (1, H), F32, kind="ExternalInput")
    if flags["lnb"]:
        lnb_d = nc.dram_tensor("lnb", (1, H), F32, kind="ExternalInput")
    if flags["bout"]:
        bout_d = nc.dram_tensor("bout", (1, P), BF16, kind="ExternalInput")

    out_d = nc.dram_tensor("out", (BS, T, P), F32, kind="ExternalOutput")

    with tile.TileContext(nc) as tc:
        with (
            tc.tile_pool(name="const", bufs=1) as cp,
            tc.tile_pool(name="work", bufs=2) as wp,
            tc.tile_pool(name="psum", bufs=1, space="PSUM") as pp,
        ):
            # ---- constants / weights into SBUF -----------------------------
            zT = cp.tile([Z, BS], F32R)
            winitT = cp.tile([Z, H], F32R)
            whh0 = cp.tile([128, 2, 3 * H], F32R)
            whh1 = cp.tile([128, 2, 3 * H], F32R)
            wih1 = cp.tile([128, 2, 3 * H], F32R)
            wout = cp.tile([128, 2, P], BF16)
            ident = cp.tile([128, 128], F32)
            c0rz = cp.tile([1, 2 * H], F32R)
            nc.sync.dma_start(out=zT, in_=zT_d[:])
            nc.sync.dma_start(out=winitT, in_=winitT_d[:])
            nc.sync.dma_start(out=ident, in_=ident_d[:])
            nc.sync.dma_start(out=c0rz, in_=c0rz_d[:])
            nc.sync.dma_start(out=whh0, in_=whh0_d[:].transpose([1, 0, 2]))
            nc.sync.dma_start(out=whh1, in_=whh1_d[:].transpose([1, 0, 2]))
            nc.sync.dma_start(out=wih1, in_=wih1_d[:].transpose([1, 0, 2]))
            nc.sync.dma_start(out=wout, in_=wout_d[:].transpose([1, 0, 2]))

            c0n = cp.tile([64, H], F32)
            nc.sync.dma_start(out=c0n, in_=c0n_d[:].partition_broadcast(64))

            def row_tile(dram, n, w, dt=F32):
                t = cp.tile([n, w], dt)
                if n > 1:
                    nc.sync.dma_start(out=t, in_=dram[:].partition_broadcast(n))
                else:
                    nc.sync.dma_start(out=t, in_=dram[:])
                return t

            binit_t = row_tile(binit_d, 64, H) if flags["binit"] else None
            c1rz_t = row_tile(c1rz_d, 1, 2 * H, F32R) if flags["c1rz"] else None
            bhh0n_t = row_tile(bhh0n_d, 1, H, F32R) if flags["bhh0n"] else None
            bhh1n_t = row_tile(bhh1n_d, 1, H, F32R) if flags["bhh1n"] else None
            bih1n_t = row_tile(bih1n_d, 1, H, F32R) if flags["bih1n"] else None
            lng_t = row_tile(lng_d, 64, H) if flags["lng"] else None
            lnb_t = row_tile(lnb_d, 64, H) if flags["lnb"] else None
            bout_t = row_tile(bout_d, 1, P, BF16) if flags["bout"] else None

            ones_f = cp.tile([1, 128], F32)
            nc.vector.memset(ones_f, 1.0)
            ones128 = cp.tile([1, 128], F32R)
            nc.vector.tensor_copy(out=ones128, in_=ones_f)
            ones = ones128[:, 0:64]
            identb = cp.tile([128, 128], BF16)
            nc.vector.tensor_copy(out=identb, in_=ident)
            onesb = None
            if flags["bout"]:
                onesb = cp.tile([1, 128], BF16)
                nc.vector.tensor_copy(out=onesb, in_=ones_f)

            # ---- helpers ----------------------------------------------------
            def elu(dst, src, width):
                """dst = elu(src) = relu(src) + exp(min(src,0)) - 1."""
                mf = wp.tile([64, H], F32, tag="elu_m", bufs=2)
                pf = wp.tile([64, H], F32, tag="elu_p", bufs=2)
                ef = wp.tile([64, H], F32, tag="elu_e", bufs=2)
                m, p_, e = mf[:, :width], pf[:, :width], ef[:, :width]
                nc.vector.tensor_scalar(out=m, in0=src, scalar1=0.0,
                                        scalar2=None, op0=ALU.min, op1=ALU.bypass)
                nc.scalar.activation(out=p_, in_=src, func=AF.Relu)
                nc.scalar.activation(out=e, in_=m, func=AF.Exp)
                nc.vector.scalar_tensor_tensor(out=dst, in0=e, scalar=-1.0,
                                               in1=p_, op0=ALU.add, op1=ALU.add)

            def emit_head(yT_pair, pair):
                """Head matmuls + staging + SWDGE DMA for timestep pair."""
                yT0 = yT_pair[:, 0].rearrange("p a b -> p (a b)")
                yT1 = yT_pair[:, 1].rearrange("p a b -> p (a b)")
                # One [128, 10000] staging tile per pair: 40KB/partition rows.
                # HWDGE store dispatch is per-descriptor-paced, so 40KB
                # descriptors lift per-queue rate; SWDGE caps ~130 GB/s on
                # 10KB column slices. Split pairs 5:3 HWDGE-duo : SWDGE.
                stg = wp.tile([128, P], F32, tag="stg", bufs=2)
                for n in range(P // NCH):
                    hp = pp.tile([128, NCH], F32, tag="head", bufs=2)
                    nc.tensor.matmul(hp, yT0, wout[:, 0, n * NCH:(n + 1) * NCH],
                                     start=True, stop=False)
                    nc.tensor.matmul(hp, yT1, wout[:, 1, n * NCH:(n + 1) * NCH],
                                     start=False, stop=not flags["bout"])
                    if flags["bout"]:
                        nc.tensor.matmul(hp, onesb,
                                         bout_t[:, n * NCH:(n + 1) * NCH],
                                         start=False, stop=True)
                    dst = stg[:, n * NCH:(n + 1) * NCH]
                    if n % 2 == 0:
                        nc.vector.tensor_copy(out=dst, in_=hp)
                    else:
                        nc.scalar.copy(out=dst, in_=hp)
                # Split every pair across all three store lanes so their caps
                # add: SWDGE takes vocab [0,4000) as two [128,2000] slices
                # (~130 GB/s lane), the HWDGE duo takes vocab [4000,10000) as
                # per-timestep [64,6000] spray slices (~170 GB/s lane).
                for q in range(2):
                    dram_ap = out_d[:, 2 * pair:2 * pair + 2,
                                    q * 2000:(q + 1) * 2000].transpose([1, 0, 2])
                    nc.gpsimd.dma_start(
                        out=dram_ap, in_=stg[:, q * 2000:(q + 1) * 2000])
                for q in range(2):
                    t = 2 * pair + q
                    eng = nc.sync if q == 0 else nc.scalar
                    eng.dma_start(out=out_d[:, t, 4000:P],
                                  in_=stg[64 * q:64 * q + 64, 4000:P])

            def emit_ypair(hist, pair):
                """Y-path for one timestep pair: LN stats + rsqrt, LN apply +
                ELU, bf16 transposes, head. Emitted every 2 steps so drain
                copies and store DMAs flow continuously."""
                st6 = wp.tile([64, 2, 6], F32, tag="st6")
                mv = wp.tile([64, 2, 2], F32, tag="mv")
                for i in range(2):
                    nc.vector.bn_stats(out=st6[:, i, :], in_=hist[:, i, :])
                    nc.vector.bn_aggr(out=mv[:, i, :], in_=st6[:, i, :])
                ve = wp.tile([64, 2], F32, tag="ve")
                nc.gpsimd.tensor_scalar(out=ve, in0=mv[:, :, 1], scalar1=LN_EPS,
                                        scalar2=None, op0=ALU.add, op1=ALU.bypass)
                yi = wp.tile([64, 2], I32, tag="yi")
                nc.vector.tensor_scalar(out=yi, in0=ve.bitcast(I32), scalar1=1,
                                        scalar2=None, op0=ALU.logical_shift_right,
                                        op1=ALU.bypass)
                nc.vector.tensor_scalar(out=yi, in0=yi, scalar1=-1,
                                        scalar2=0x5F3759DF, op0=ALU.mult,
                                        op1=ALU.add)
                rs = yi.bitcast(F32)
                tn = wp.tile([64, 2], F32, tag="tn")
                for _ in range(RSQRT_NEWTON):
                    nc.gpsimd.tensor_tensor(out=tn, in0=rs, in1=rs, op=ALU.mult)
                    nc.gpsimd.tensor_tensor(out=tn, in0=tn, in1=ve, op=ALU.mult)
                    nc.gpsimd.tensor_scalar(out=tn, in0=tn, scalar1=-0.5,
                                            scalar2=1.5, op0=ALU.mult, op1=ALU.add)
                    nc.gpsimd.tensor_tensor(out=rs, in0=rs, in1=tn, op=ALU.mult)
                # per-step LN apply + ELU, in place on hist
                for i in range(2):
                    nc.vector.tensor_scalar(out=hist[:, i, :], in0=hist[:, i, :],
                                            scalar1=mv[:, i, 0:1],
                                            scalar2=rs[:, i:i + 1],
                                            op0=ALU.subtract, op1=ALU.mult)
                    if flags["lng"]:
                        nc.vector.tensor_tensor(out=hist[:, i, :], in0=hist[:, i, :],
                                                in1=lng_t, op=ALU.mult)
                    if flags["lnb"]:
                        nc.vector.tensor_tensor(out=hist[:, i, :], in0=hist[:, i, :],
                                                in1=lnb_t, op=ALU.add)
                    elu(hist[:, i, :], hist[:, i, :], H)
                # bf16 cast + transpose + head
                histb = wp.tile([64, 2, H], BF16, tag="histb", bufs=2)
                nc.vector.tensor_copy(
                    out=histb.rearrange("p a h -> p (a h)"),
                    in_=hist.rearrange("p a h -> p (a h)"))
                typ = pp.tile([128, 256], BF16, tag="tpb", bufs=1)
                for par in range(2):
                    for c in range(2):
                        nc.tensor.transpose(
                            typ[:, c * 128 + par * 64: c * 128 + (par + 1) * 64],
                            histb[:, par, c * 128:(c + 1) * 128],
                            identb[0:64, 0:64])
                yT_pair = wp.tile([128, 2, 2, 64], BF16, tag="yT", bufs=2)
                nc.vector.tensor_copy(
                    out=yT_pair.rearrange("p c a b -> p (c a b)"),
                    in_=typ[:, 0:256])
                emit_head(yT_pair, pair)

            # ---- init: h0 = elu(z @ W_init.T + b_init) ----------------------
            itp = pp.tile([128, 384], F32, tag="tp", bufs=1)
            nc.tensor.matmul(itp[0:64, 0:H], zT, winitT, start=True, stop=True)
            h0pre = wp.tile([64, H], F32, tag="h0pre")
            if flags["binit"]:
                nc.vector.tensor_tensor(out=h0pre, in0=itp[0:64, 0:H],
                                        in1=binit_t, op=ALU.add)
            else:
                nc.vector.tensor_copy(out=h0pre, in_=itp[0:64, 0:H])
            hh_prev = wp.tile([64, 2, H], F32, tag="hh", bufs=3)
            elu(hh_prev[:, 0, :], h0pre, H)
            nc.vector.tensor_copy(out=hh_prev[:, 1, :], in_=hh_prev[:, 0, :])
            # transpose init state -> hT_prev [128, 2(c), 2(l), 64(b)]
            itp2 = pp.tile([128, 384], F32, tag="tp", bufs=1)
            for l in range(2):
                for c in range(2):
                    nc.tensor.transpose(
                        itp2[:, c * 128 + l * 64: c * 128 + (l + 1) * 64],
                        hh_prev[:, l, c * 128:(c + 1) * 128], ident[0:64, 0:64])
            hT_prev = wp.tile([128, 2, 2, 64], F32R, tag="hT", bufs=3)
            nc.vector.tensor_copy(out=hT_prev.rearrange("p c l b -> p (c l b)"),
                                  in_=itp2[:, 0:256])

            # ---- main loop: slots 0..T --------------------------------------
            for s in range(T + 1):
                L0 = s < T     # layer-0 computes h0_s   (hh slot 1)
                L1 = s >= 1    # layer-1 computes h1_{s-1} (hh slot 0)

                hp_ctx = tc.high_priority()
                hp_ctx.__enter__()
                h1T = lambda k: hT_prev[:, k, 0, :]
                h0T = lambda k: hT_prev[:, k, 1, :]

                # gate matmuls: rz [64, 2, 512] (l1 | l0), nx [64, 768]
                rz = pp.tile([64, 2, 2 * H], F32, tag="rz", bufs=1)
                nx = pp.tile([64, 3 * H], F32, tag="nx", bufs=1)
                if L0:
                    nc.tensor.matmul(rz[:, 1, :], h0T(0), whh0[:, 0, 0:2 * H],
                                     start=True, stop=False)
                    nc.tensor.matmul(rz[:, 1, :], h0T(1), whh0[:, 1, 0:2 * H],
                                     start=False, stop=False)
                    nc.tensor.matmul(rz[:, 1, :], ones, c0rz,
                                     start=False, stop=True)
                    nc.tensor.matmul(nx[:, 2 * H:], h0T(0), whh0[:, 0, 2 * H:],
                                     start=True, stop=False)
                    nc.tensor.matmul(nx[:, 2 * H:], h0T(1), whh0[:, 1, 2 * H:],
                                     start=False, stop=not flags["bhh0n"])
                    if flags["bhh0n"]:
                        nc.tensor.matmul(nx[:, 2 * H:], ones, bhh0n_t,
                                         start=False, stop=True)
                if L1:
                    nc.tensor.matmul(rz[:, 0, :], h1T(0), whh1[:, 0, 0:2 * H],
                                     start=True, stop=False)
                    nc.tensor.matmul(rz[:, 0, :], h1T(1), whh1[:, 1, 0:2 * H],
                                     start=False, stop=False)
                    nc.tensor.matmul(rz[:, 0, :], h0T(0), wih1[:, 0, 0:2 * H],
                                     start=False, stop=False)
                    nc.tensor.matmul(rz[:, 0, :], h0T(1), wih1[:, 1, 0:2 * H],
                                     start=False, stop=not flags["c1rz"])
                    if flags["c1rz"]:
                        nc.tensor.matmul(rz[:, 0, :], ones, c1rz_t,
                                         start=False, stop=True)
                    nc.tensor.matmul(nx[:, 0:H], h1T(0), whh1[:, 0, 2 * H:],
                                     start=True, stop=False)
                    nc.tensor.matmul(nx[:, 0:H], h1T(1), whh1[:, 1, 2 * H:],
                                     start=False, stop=not flags["bhh1n"])
                    if flags["bhh1n"]:
                        nc.tensor.matmul(nx[:, 0:H], ones, bhh1n_t,
                                         start=False, stop=True)
                    nc.tensor.matmul(nx[:, H:2 * H], h0T(0), wih1[:, 0, 2 * H:],
                                     start=True, stop=False)
                    nc.tensor.matmul(nx[:, H:2 * H], h0T(1), wih1[:, 1, 2 * H:],
                                     start=False, stop=not flags["bih1n"])
                    if flags["bih1n"]:
                        nc.tensor.matmul(nx[:, H:2 * H], ones, bih1n_t,
                                         start=False, stop=True)

                # gates via tanh: sigmoid(x) = 0.5 + 0.5*tanh(x/2)
                tru = wp.tile([64, 2, 2 * H], F32, tag="ru")
                rr = wp.tile([64, 2, H], F32, tag="rr")
                tt = wp.tile([64, 2, H], F32, tag="tt")
                aa = wp.tile([64, 2, H], F32, tag="aa")
                nn = wp.tile([64, 2, H], F32, tag="nn")
                uu = wp.tile([64, 2, H], F32, tag="uu")
                dd = wp.tile([64, 2, H], F32, tag="tt")
                vv = wp.tile([64, 2, H], F32, tag="vv")
                mm_ = wp.tile([64, 2, H], F32, tag="aa")
                hh_new = wp.tile([64, 2, H], F32, tag="hh", bufs=3)
                # hn columns per layer: l1 -> nx[:, 0:H], l0 -> nx[:, 2H:3H]
                hn = nx.rearrange("p (a h) -> p a h", a=3)

                active = ([0] if L1 else []) + ([1] if L0 else [])
                if len(active) == 2:
                    nc.scalar.activation(out=tru.rearrange("p l w -> p (l w)"),
                                         in_=rz.rearrange("p l w -> p (l w)"),
                                         func=AF.Tanh, scale=0.5)
                    # r for both layers, then r*hn (hn slots (0, 2) strided)
                    nc.vector.tensor_scalar(out=rr, in0=tru[:, :, 0:H],
                                            scalar1=0.5, scalar2=0.5,
                                            op0=ALU.mult, op1=ALU.add)
                    hnview = nx.rearrange("p (a h) -> p a h", a=3)[:, 0::2, :]
                    nc.vector.tensor_tensor(out=tt, in0=rr, in1=hnview,
                                            op=ALU.mult)
                else:
                    l = active[0]
                    nc.scalar.activation(out=tru[:, l], in_=rz[:, l],
                                         func=AF.Tanh, scale=0.5)
                    nc.vector.tensor_scalar(out=rr[:, l], in0=tru[:, l, 0:H],
                                            scalar1=0.5, scalar2=0.5,
                                            op0=ALU.mult, op1=ALU.add)
                    nc.vector.tensor_tensor(out=tt[:, l], in0=rr[:, l],
                                            in1=hn[:, 2 * l], op=ALU.mult)
                if L1:
                    nc.vector.tensor_tensor(out=aa[:, 0], in0=tt[:, 0],
                                            in1=nx[:, H:2 * H], op=ALU.add)
                if L0:
                    nc.vector.tensor_tensor(out=aa[:, 1], in0=tt[:, 1],
                                            in1=c0n, op=ALU.add)

                if len(active) == 2:
                    # u = 0.5+0.5*t_z ; dd = u*h ; v = 1-u = 0.5-0.5*t_z
                    nc.vector.tensor_scalar(out=uu, in0=tru[:, :, H:2 * H],
                                            scalar1=0.5, scalar2=0.5,
                                            op0=ALU.mult, op1=ALU.add)
                    nc.vector.tensor_tensor(out=dd, in0=uu,
                                            in1=hh_prev, op=ALU.mult)
                    nc.vector.tensor_scalar(out=vv, in0=tru[:, :, H:2 * H],
                                            scalar1=-0.5, scalar2=0.5,
                                            op0=ALU.mult, op1=ALU.add)
                    nc.scalar.activation(out=nn.rearrange("p l h -> p (l h)"),
                                         in_=aa.rearrange("p l h -> p (l h)"),
                                         func=AF.Tanh)
                    nc.vector.tensor_tensor(out=mm_, in0=vv, in1=nn, op=ALU.mult)
                    nc.vector.tensor_tensor(out=hh_new, in0=dd, in1=mm_,
                                            op=ALU.add)
                else:
                    l = active[0]
                    nc.vector.tensor_scalar(out=uu[:, l], in0=tru[:, l, H:2 * H],
                                            scalar1=0.5, scalar2=0.5,
                                            op0=ALU.mult, op1=ALU.add)
                    nc.vector.tensor_tensor(out=dd[:, l], in0=uu[:, l],
                                            in1=hh_prev[:, l], op=ALU.mult)
                    nc.vector.tensor_scalar(out=vv[:, l], in0=tru[:, l, H:2 * H],
                                            scalar1=-0.5, scalar2=0.5,
                                            op0=ALU.mult, op1=ALU.add)
                    nc.scalar.activation(out=nn[:, l], in_=aa[:, l], func=AF.Tanh)
                    nc.vector.tensor_tensor(out=mm_[:, l], in0=vv[:, l],
                                            in1=nn[:, l], op=ALU.mult)
                    nc.vector.tensor_tensor(out=hh_new[:, l], in0=dd[:, l],
                                            in1=mm_[:, l], op=ALU.add)
                if s == 0:
                    nc.vector.tensor_copy(out=hh_new[:, 0], in_=hh_prev[:, 0])

                tp = pp.tile([128, 384], F32, tag="tp", bufs=1)

                # defer y-path: copy h1_{s-1} into the pair history buffer
                if L1:
                    i = (s - 1) % 2
                    if i == 0:
                        hist = wp.tile([64, 2, H], F32, tag="hist", bufs=2)
                    nc.scalar.copy(out=hist[:, i, :], in_=hh_new[:, 0])

                # state transposes -> tp[:, 0:256], layout (c, l, b)
                if L0:
                    lset = (0, 1) if L1 else (1,)
                    for c in range(2):
                        for l in lset:
                            nc.tensor.transpose(
                                tp[:, c * 128 + l * 64: c * 128 + (l + 1) * 64],
                                hh_new[:, l, c * 128:(c + 1) * 128],
                                ident[0:64, 0:64])
                    hT_new = wp.tile([128, 2, 2, 64], F32R, tag="hT", bufs=3)
                    if L1:
                        nc.vector.tensor_copy(
                            out=hT_new.rearrange("p c l b -> p (c l b)"),
                            in_=tp[:, 0:256])
                    else:
                        # slot 0: layer-1 half of hT carries the init state
                        nc.vector.tensor_copy(
                            out=hT_new[:, :, 1, :],
                            in_=tp[:, 0:256]
                            .rearrange("p (c b) -> p c b", c=2)[:, :, 64:128])
                        nc.vector.tensor_copy(
                            out=hT_new[:, :, 0, :],
                            in_=hT_prev[:, :, 0, :])
                else:
                    hT_new = hT_prev

                hp_ctx.__exit__(None, None, None)
                if L1 and (s - 1) % 2 == 1:
                    emit_ypair(hist, (s - 1) // 2)

                hh_prev = hh_new
                hT_prev = hT_new

    nc.compile()
    return nc


_cache = {}


def _get_program(flags):
    key = tuple(sorted(flags.items()))
    if key not in _cache:
        _cache[key] = _build(flags)
    return _cache[key]


def kernel(z, W_init, b_init, embedding, W_ih0, W_hh0, b_ih0, b_hh0,
           W_ih1, W_hh1, b_ih1, b_hh1, ln_g, ln_b, W_out, b_out):
    global last_exec_ns, last_results
    z = _np(z); W_init = _np(W_init); b_init = _np(b_init)
    embedding = _np(embedding)
    W_ih0 = _np(W_ih0); W_hh0 = _np(W_hh0); b_ih0 = _np(b_ih0); b_hh0 = _np(b_hh0)
    W_ih1 = _np(W_ih1); W_hh1 = _np(W_hh1); b_ih1 = _np(b_ih1); b_hh1 = _np(b_hh1)
    ln_g = _np(ln_g); ln_b = _np(ln_b); W_out = _np(W_out); b_out = _np(b_out)

    # layer-0 input gates are constant across (b, t): fold embedding @ W_ih0.T
    gx0 = (embedding @ W_ih0.T + b_ih0).reshape(1, 3 * H)
    c0rz = gx0[:, 0:2 * H] + b_hh0[None, 0:2 * H]
    c0n = gx0[:, 2 * H:]
    c1rz = (b_ih1 + b_hh1)[None, 0:2 * H]

    flags = {
        "binit": bool(np.any(b_init != 0)),
        "c1rz": bool(np.any(c1rz != 0)),
        "bhh0n": bool(np.any(b_hh0[2 * H:] != 0)),
        "bhh1n": bool(np.any(b_hh1[2 * H:] != 0)),
        "bih1n": bool(np.any(b_ih1[2 * H:] != 0)),
        "lng": bool(np.any(ln_g != 1.0)),
        "lnb": bool(np.any(ln_b != 0)),
        "bout": bool(np.any(b_out != 0)),
    }
    nc = _get_program(flags)

    common = {
        "winitT": np.ascontiguousarray(W_init.T),
        "whh0T": np.ascontiguousarray(W_hh0.T.reshape(2, 128, 3 * H)),
        "whh1T": np.ascontiguousarray(W_hh1.T.reshape(2, 128, 3 * H)),
        "wih1T": np.ascontiguousarray(W_ih1.T.reshape(2, 128, 3 * H)),
        "woutT": np.ascontiguousarray(W_out.T.reshape(2, 128, P))
            .astype(ml_dtypes.bfloat16),
        "ident": np.eye(128, dtype=np.float32),
        "c0rz": np.ascontiguousarray(c0rz),
        "c0n": np.ascontiguousarray(c0n),
    }
    if flags["binit"]:
        common["binit"] = b_init.reshape(1, H)
    if flags["c1rz"]:
        common["c1rz"] = np.ascontiguousarray(c1rz)
    if flags["bhh0n"]:
        common["bhh0n"] = np.ascontiguousarray(b_hh0[None, 2 * H:])
    if flags["bhh1n"]:
        common["bhh1n"] = np.ascontiguousarray(b_hh1[None, 2 * H:])
    if flags["bih1n"]:
        common["bih1n"] = np.ascontiguousarray(b_ih1[None, 2 * H:])
    if flags["lng"]:
        common["lng"] = ln_g.reshape(1, H)
    if flags["lnb"]:
        common["lnb"] = ln_b.reshape(1, H)
    if flags["bout"]:
        common["bout"] = b_out.reshape(1, P).astype(ml_dtypes.bfloat16)

    in_maps = []
    for c in range(NCORES):
        m = dict(common)
        m["zT"] = np.ascontiguousarray(z[c * BS:(c + 1) * BS].T)
        in_maps.append(m)

    trace = os.environ.get("KERNEL_TRACE", "0") == "1"
    res = run_bass_kernel_spmd(nc, in_maps, core_ids=list(range(NCORES)),
                               trace=trace)
    last_exec_ns = res.exec_time_ns
    last_results = res
    return np.concatenate([r["out"][None] for r in res.results], axis=0) \
             .reshape(B, T, P)


# revision 29
# speedup vs baseline: 1.1582x; 1.1582x over previous
"""Trainium2 Bass kernel for nn_Decoder: 2-layer GRU decoder + LayerNorm + ELU + vocab head.

Contract: kernel(**inputs) takes the FULL unsharded inputs (as produced by the
reference setup_inputs) and returns the FULL (512, 64, 10000) float32 logits.
Internally: data-parallel shard of batch B=512 across 8 NeuronCores; all
weights replicated. Self-contained (shapes hardcoded).

Layout notes (per core, BS=64 batch rows):
- Both GRU layers run interleaved, one slot apart: slot s computes layer-0
  step s and layer-1 step s-1. State tile hh [64, 2, 256] = (b, layer, h)
  with layer 1 in slot 0. GRU matmuls use float32r (full PE rate).
- Output stores go through the gpsimd SWDGE queue: HWDGE (sync/scalar)
  SBUF->DRAM stores serialize onto 2 SDMA engines (~26 GB/s/queue), while
  SWDGE fans descriptors across all 16 engines (~358 GB/s HBM-bound).
  Each store is [128, 5000] f32 = 2.56 MB with 20 KB/partition descriptors.
- Head matmuls in bf16 (yT stationary, W_out.T moving): 2 cols/cycle on the
  PE + half the weight-load traffic. PSUM accumulates f32, logits exact
  enough (~1e-3 rel).
- ACT stays on the exp_and_others table set the whole kernel: tanh for GRU
  gates (sigmoid(x) = 0.5 + 0.5*tanh(x/2)), exp for ELU
  (elu(x) = max(x,0) + exp(min(x,0)) - 1). LN rsqrt = bit-trick + Newton
  (on GpSimd).
"""
import os
import sys

for _p in ("/opt/trn_rl_repo", "/root/.axon_site/_ro/trn_rl_repo"):
    if os.path.isdir(_p) and _p not in sys.path:
        sys.path.append(_p)

import numpy as np
import ml_dtypes
import concourse.bacc as bacc
import concourse.mybir as mybir
import concourse.tile as tile
from concourse.bass_utils import run_bass_kernel_spmd

F32 = mybir.dt.float32
F32R = mybir.dt.float32r
BF16 = mybir.dt.bfloat16
I32 = mybir.dt.int32
AF = mybir.ActivationFunctionType
ALU = mybir.AluOpType

B, Z, H, T, P = 512, 64, 256, 64, 10000
NCORES = 8
BS = B // NCORES
LN_EPS = 1e-5
NCH = 500                 # head N-chunk (PSUM bank = 512 f32; 20*500 = P)
NSTG = 2500               # staging tile cols (vocab quarter, 10KB descs)
RSQRT_NEWTON = 3
G = 8                     # y-path batch (steps)

last_exec_ns = None
last_results = None


def _np(x):
    return np.ascontiguousarray(np.asarray(x, dtype=np.float32))


def _build(flags):
    nc = bacc.Bacc("TRN2", target_bir_lowering=False)

    zT_d = nc.dram_tensor("zT", (Z, BS), F32R, kind="ExternalInput")
    winitT_d = nc.dram_tensor("winitT", (Z, H), F32R, kind="ExternalInput")
    whh0_d = nc.dram_tensor("whh0T", (2, 128, 3 * H), F32R, kind="ExternalInput")
    whh1_d = nc.dram_tensor("whh1T", (2, 128, 3 * H), F32R, kind="ExternalInput")
    wih1_d = nc.dram_tensor("wih1T", (2, 128, 3 * H), F32R, kind="ExternalInput")
    wout_d = nc.dram_tensor("woutT", (2, 128, P), BF16, kind="ExternalInput")
    ident_d = nc.dram_tensor("ident", (128, 128), F32, kind="ExternalInput")
    c0rz_d = nc.dram_tensor("c0rz", (1, 2 * H), F32R, kind="ExternalInput")
    c0n_d = nc.dram_tensor("c0n", (1, H), F32, kind="ExternalInput")
    if flags["binit"]:
        binit_d = nc.dram_tensor("binit", (1, H), F32, kind="ExternalInput")
    if flags["c1rz"]:
        c1rz_d = nc.dram_tensor("c1rz", (1, 2 * H), F32R, kind="ExternalInput")
    if flags["bhh0n"]:
        bhh0n_d = nc.dram_tensor("bhh0n", (1, H), F32R, kind="ExternalInput")
    if flags["bhh1n"]:
        bhh1n_d = nc.dram_tensor("bhh1n", (1, H), F32R, kind="ExternalInput")
    if flags["bih1n"]:
        bih1n_d = nc.dram_tensor("bih1n", (1, H), F32R, kind="ExternalInput")
    if flags["lng"]:
        lng_d = nc.dram_tensor("lng", (1, H), F32, kind="ExternalInput")
    if flags["lnb"]:
        lnb_d = nc.dram_tensor("lnb", (1, H), F32, kind="ExternalInput")
    if flags["bout"]:
        bout_d = nc.dram_tensor("bout", (1, P), BF16, kind="ExternalInput")

    out_d = nc.dram_tensor("out", (BS, T, P), F32, kind="ExternalOutput")

    with tile.TileContext(nc) as tc:
        with (
            tc.tile_pool(name="const", bufs=1) as cp,
            tc.tile_pool(name="work", bufs=2) as wp,
            tc.tile_pool(name="psum", bufs=1, space="PSUM") as pp,
        ):
            # ---- constants / weights into SBUF -----------------------------
            zT = cp.tile([Z, BS], F32R)
            winitT = cp.tile([Z, H], F32R)
            whh0 = cp.tile([128, 2, 3 * H], F32R)
            whh1 = cp.tile([128, 2, 3 * H], F32R)
            wih1 = cp.tile([128, 2, 3 * H], F32R)
            wout = cp.tile([128, 2, P], BF16)
            ident = cp.tile([128, 128], F32)
            c0rz = cp.tile([1, 2 * H], F32R)
            nc.sync.dma_start(out=zT, in_=zT_d[:])
            nc.sync.dma_start(out=winitT, in_=winitT_d[:])
            nc.sync.dma_start(out=ident, in_=ident_d[:])
            nc.sync.dma_start(out=c0rz, in_=c0rz_d[:])
            nc.sync.dma_start(out=whh0, in_=whh0_d[:].transpose([1, 0, 2]))
            nc.sync.dma_start(out=whh1, in_=whh1_d[:].transpose([1, 0, 2]))
            nc.sync.dma_start(out=wih1, in_=wih1_d[:].transpose([1, 0, 2]))
            nc.sync.dma_start(out=wout, in_=wout_d[:].transpose([1, 0, 2]))

            c0n = cp.tile([64, H], F32)
            nc.sync.dma_start(out=c0n, in_=c0n_d[:].partition_broadcast(64))

            def row_tile(dram, n, w, dt=F32):
                t = cp.tile([n, w], dt)
                if n > 1:
                    nc.sync.dma_start(out=t, in_=dram[:].partition_broadcast(n))
                else:
                    nc.sync.dma_start(out=t, in_=dram[:])
                return t

            binit_t = row_tile(binit_d, 64, H) if flags["binit"] else None
            c1rz_t = row_tile(c1rz_d, 1, 2 * H, F32R) if flags["c1rz"] else None
            bhh0n_t = row_tile(bhh0n_d, 1, H, F32R) if flags["bhh0n"] else None
            bhh1n_t = row_tile(bhh1n_d, 1, H, F32R) if flags["bhh1n"] else None
            bih1n_t = row_tile(bih1n_d, 1, H, F32R) if flags["bih1n"] else None
            lng_t = row_tile(lng_d, 64, H) if flags["lng"] else None
            lnb_t = row_tile(lnb_d, 64, H) if flags["lnb"] else None
            bout_t = row_tile(bout_d, 1, P, BF16) if flags["bout"] else None

            ones_f = cp.tile([1, 128], F32)
            nc.vector.memset(ones_f, 1.0)
            ones128 = cp.tile([1, 128], F32R)
            nc.vector.tensor_copy(out=ones128, in_=ones_f)
            ones = ones128[:, 0:64]
            identb = cp.tile([128, 128], BF16)
            nc.vector.tensor_copy(out=identb, in_=ident)
            onesb = None
            if flags["bout"]:
                onesb = cp.tile([1, 128], BF16)
                nc.vector.tensor_copy(out=onesb, in_=ones_f)

            # ---- helpers ----------------------------------------------------
            def elu(dst, src, width):
                """dst = elu(src) = relu(src) + exp(min(src,0)) - 1."""
                mf = wp.tile([64, H], F32, tag="elu_m", bufs=2)
                pf = wp.tile([64, H], F32, tag="elu_p", bufs=2)
                ef = wp.tile([64, H], F32, tag="elu_e", bufs=2)
                m, p_, e = mf[:, :width], pf[:, :width], ef[:, :width]
                nc.vector.tensor_scalar(out=m, in0=src, scalar1=0.0,
                                        scalar2=None, op0=ALU.min, op1=ALU.bypass)
                nc.scalar.activation(out=p_, in_=src, func=AF.Relu)
                nc.scalar.activation(out=e, in_=m, func=AF.Exp)
                nc.vector.scalar_tensor_tensor(out=dst, in0=e, scalar=-1.0,
                                               in1=p_, op0=ALU.add, op1=ALU.add)

            def emit_head(yT_pair, pair):
                """Head matmuls + staging + SWDGE DMA for timestep pair."""
                yT0 = yT_pair[:, 0].rearrange("p a b -> p (a b)")
                yT1 = yT_pair[:, 1].rearrange("p a b -> p (a b)")
                # Store scheme (microbenched at ~305 GB/s/core): per vocab
                # quarter, [64, 2500] slices with sync always storing t0
                # (partitions 0-63, even SBUF ports) and scalar always t1
                # (odd ports), so the two HWDGE queues drive disjoint port
                # sets concurrently. Quarter-granular staging tiles let each
                # quarter's stores launch as soon as its 5 chunks drain.
                for q in range(4):
                    stg = wp.tile([128, NSTG], F32, tag="stg", bufs=8)
                    for j in range(NSTG // NCH):
                        n = q * (NSTG // NCH) + j
                        hp = pp.tile([128, NCH], F32, tag="head", bufs=2)
                        nc.tensor.matmul(hp, yT0, wout[:, 0, n * NCH:(n + 1) * NCH],
                                         start=True, stop=False)
                        nc.tensor.matmul(hp, yT1, wout[:, 1, n * NCH:(n + 1) * NCH],
                                         start=False, stop=not flags["bout"])
                        if flags["bout"]:
                            nc.tensor.matmul(hp, onesb,
                                             bout_t[:, n * NCH:(n + 1) * NCH],
                                             start=False, stop=True)
                        dst = stg[:, j * NCH:(j + 1) * NCH]
                        if j % 2 == 0:
                            nc.vector.tensor_copy(out=dst, in_=hp)
                        else:
                            nc.scalar.copy(out=dst, in_=hp)
                    v = slice(q * NSTG, (q + 1) * NSTG)
                    nc.sync.dma_start(out=out_d[:, 2 * pair, v],
                                      in_=stg[0:64, :])
                    nc.scalar.dma_start(out=out_d[:, 2 * pair + 1, v],
                                        in_=stg[64:128, :])

            def emit_ypair(hist, pair):
                """Y-path for one timestep pair: LN stats + rsqrt, LN apply +
                ELU, bf16 transposes, head. Emitted every 2 steps so drain
                copies and store DMAs flow continuously."""
                st6 = wp.tile([64, 2, 6], F32, tag="st6")
                mv = wp.tile([64, 2, 2], F32, tag="mv")
                for i in range(2):
                    nc.vector.bn_stats(out=st6[:, i, :], in_=hist[:, i, :])
                    nc.vector.bn_aggr(out=mv[:, i, :], in_=st6[:, i, :])
                ve = wp.tile([64, 2], F32, tag="ve")
                nc.gpsimd.tensor_scalar(out=ve, in0=mv[:, :, 1], scalar1=LN_EPS,
                                        scalar2=None, op0=ALU.add, op1=ALU.bypass)
                yi = wp.tile([64, 2], I32, tag="yi")
                nc.vector.tensor_scalar(out=yi, in0=ve.bitcast(I32), scalar1=1,
                                        scalar2=None, op0=ALU.logical_shift_right,
                                        op1=ALU.bypass)
                nc.vector.tensor_scalar(out=yi, in0=yi, scalar1=-1,
                                        scalar2=0x5F3759DF, op0=ALU.mult,
                                        op1=ALU.add)
                rs = yi.bitcast(F32)
                tn = wp.tile([64, 2], F32, tag="tn")
                for _ in range(RSQRT_NEWTON):
                    nc.gpsimd.tensor_tensor(out=tn, in0=rs, in1=rs, op=ALU.mult)
                    nc.gpsimd.tensor_tensor(out=tn, in0=tn, in1=ve, op=ALU.mult)
                    nc.gpsimd.tensor_scalar(out=tn, in0=tn, scalar1=-0.5,
                                            scalar2=1.5, op0=ALU.mult, op1=ALU.add)
                    nc.gpsimd.tensor_tensor(out=rs, in0=rs, in1=tn, op=ALU.mult)
                # per-step LN apply + ELU, in place on hist
                for i in range(2):
                    nc.vector.tensor_scalar(out=hist[:, i, :], in0=hist[:, i, :],
                                            scalar1=mv[:, i, 0:1],
                                            scalar2=rs[:, i:i + 1],
                                            op0=ALU.subtract, op1=ALU.mult)
                    if flags["lng"]:
                        nc.vector.tensor_tensor(out=hist[:, i, :], in0=hist[:, i, :],
                                                in1=lng_t, op=ALU.mult)
                    if flags["lnb"]:
                        nc.vector.tensor_tensor(out=hist[:, i, :], in0=hist[:, i, :],
                                                in1=lnb_t, op=ALU.add)
                    elu(hist[:, i, :], hist[:, i, :], H)
                # bf16 cast + transpose + head
                histb = wp.tile([64, 2, H], BF16, tag="histb", bufs=2)
                nc.vector.tensor_copy(
                    out=histb.rearrange("p a h -> p (a h)"),
                    in_=hist.rearrange("p a h -> p (a h)"))
                typ = pp.tile([128, 256], BF16, tag="tpb", bufs=1)
                for par in range(2):
                    for c in range(2):
                        nc.tensor.transpose(
                            typ[:, c * 128 + par * 64: c * 128 + (par + 1) * 64],
                            histb[:, par, c * 128:(c + 1) * 128],
                            identb[0:64, 0:64])
                yT_pair = wp.tile([128, 2, 2, 64], BF16, tag="yT", bufs=2)
                nc.vector.tensor_copy(
                    out=yT_pair.rearrange("p c a b -> p (c a b)"),
                    in_=typ[:, 0:256])
                emit_head(yT_pair, pair)

            # ---- init: h0 = elu(z @ W_init.T + b_init) ----------------------
            itp = pp.tile([128, 384], F32, tag="tp", bufs=1)
            nc.tensor.matmul(itp[0:64, 0:H], zT, winitT, start=True, stop=True)
            h0pre = wp.tile([64, H], F32, tag="h0pre")
            if flags["binit"]:
                nc.vector.tensor_tensor(out=h0pre, in0=itp[0:64, 0:H],
                                        in1=binit_t, op=ALU.add)
            else:
                nc.vector.tensor_copy(out=h0pre, in_=itp[0:64, 0:H])
            hh_prev = wp.tile([64, 2, H], F32, tag="hh", bufs=3)
            elu(hh_prev[:, 0, :], h0pre, H)
            nc.vector.tensor_copy(out=hh_prev[:, 1, :], in_=hh_prev[:, 0, :])
            # transpose init state -> hT_prev [128, 2(c), 2(l), 64(b)]
            itp2 = pp.tile([128, 384], F32, tag="tp", bufs=1)
            for l in range(2):
                for c in range(2):
                    nc.tensor.transpose(
                        itp2[:, c * 128 + l * 64: c * 128 + (l + 1) * 64],
                        hh_prev[:, l, c * 128:(c + 1) * 128], ident[0:64, 0:64])
            hT_prev = wp.tile([128, 2, 2, 64], F32R, tag="hT", bufs=3)
            nc.vector.tensor_copy(out=hT_prev.rearrange("p c l b -> p (c l b)"),
                                  in_=itp2[:, 0:256])

            # ---- main loop: slots 0..T --------------------------------------
            for s in range(T + 1):
                L0 = s < T     # layer-0 computes h0_s   (hh slot 1)
                L1 = s >= 1    # layer-1 computes h1_{s-1} (hh slot 0)

                hp_ctx = tc.high_priority()
                hp_ctx.__enter__()
                h1T = lambda k: hT_prev[:, k, 0, :]
                h0T = lambda k: hT_prev[:, k, 1, :]

                # gate matmuls: rz [64, 2, 512] (l1 | l0), nx [64, 768]
                rz = pp.tile([64, 2, 2 * H], F32, tag="rz", bufs=1)
                nx = pp.tile([64, 3 * H], F32, tag="nx", bufs=1)
                if L0:
                    nc.tensor.matmul(rz[:, 1, :], h0T(0), whh0[:, 0, 0:2 * H],
                                     start=True, stop=False)
                    nc.tensor.matmul(rz[:, 1, :], h0T(1), whh0[:, 1, 0:2 * H],
                                     start=False, stop=False)
                    nc.tensor.matmul(rz[:, 1, :], ones, c0rz,
                                     start=False, stop=True)
                    nc.tensor.matmul(nx[:, 2 * H:], h0T(0), whh0[:, 0, 2 * H:],
                                     start=True, stop=False)
                    nc.tensor.matmul(nx[:, 2 * H:], h0T(1), whh0[:, 1, 2 * H:],
                                     start=False, stop=not flags["bhh0n"])
                    if flags["bhh0n"]:
                        nc.tensor.matmul(nx[:, 2 * H:], ones, bhh0n_t,
                                         start=False, stop=True)
                if L1:
                    nc.tensor.matmul(rz[:, 0, :], h1T(0), whh1[:, 0, 0:2 * H],
                                     start=True, stop=False)
                    nc.tensor.matmul(rz[:, 0, :], h1T(1), whh1[:, 1, 0:2 * H],
                                     start=False, stop=False)
                    nc.tensor.matmul(rz[:, 0, :], h0T(0), wih1[:, 0, 0:2 * H],
                                     start=False, stop=False)
                    nc.tensor.matmul(rz[:, 0, :], h0T(1), wih1[:, 1, 0:2 * H],
                                     start=False, stop=not flags["c1rz"])
                    if flags["c1rz"]:
                        nc.tensor.matmul(rz[:, 0, :], ones, c1rz_t,
                                         start=False, stop=True)
                    nc.tensor.matmul(nx[:, 0:H], h1T(0), whh1[:, 0, 2 * H:],
                                     start=True, stop=False)
                    nc.tensor.matmul(nx[:, 0:H], h1T(1), whh1[:, 1, 2 * H:],
                                     start=False, stop=not flags["bhh1n"])
                    if flags["bhh1n"]:
                        nc.tensor.matmul(nx[:, 0:H], ones, bhh1n_t,
                                         start=False, stop=True)
                    nc.tensor.matmul(nx[:, H:2 * H], h0T(0), wih1[:, 0, 2 * H:],
                                     start=True, stop=False)
                    nc.tensor.matmul(nx[:, H:2 * H], h0T(1), wih1[:, 1, 2 * H:],
                                     start=False, stop=not flags["bih1n"])
                    if flags["bih1n"]:
                        nc.tensor.matmul(nx[:, H:2 * H], ones, bih1n_t,
                                         start=False, stop=True)

                # gates via tanh: sigmoid(x) = 0.5 + 0.5*tanh(x/2)
                tru = wp.tile([64, 2, 2 * H], F32, tag="ru")
                rr = wp.tile([64, 2, H], F32, tag="rr")
                tt = wp.tile([64, 2, H], F32, tag="tt")
                aa = wp.tile([64, 2, H], F32, tag="aa")
                nn = wp.tile([64, 2, H], F32, tag="nn")
                uu = wp.tile([64, 2, H], F32, tag="uu")
                dd = wp.tile([64, 2, H], F32, tag="tt")
                vv = wp.tile([64, 2, H], F32, tag="vv")
                mm_ = wp.tile([64, 2, H], F32, tag="aa")
                hh_new = wp.tile([64, 2, H], F32, tag="hh", bufs=3)
                # hn columns per layer: l1 -> nx[:, 0:H], l0 -> nx[:, 2H:3H]
                hn = nx.rearrange("p (a h) -> p a h", a=3)

                active = ([0] if L1 else []) + ([1] if L0 else [])
                if len(active) == 2:
                    nc.scalar.activation(out=tru.rearrange("p l w -> p (l w)"),
                                         in_=rz.rearrange("p l w -> p (l w)"),
                                         func=AF.Tanh, scale=0.5)
                    # r for both layers, then r*hn (hn slots (0, 2) strided)
                    nc.vector.tensor_scalar(out=rr, in0=tru[:, :, 0:H],
                                            scalar1=0.5, scalar2=0.5,
                                            op0=ALU.mult, op1=ALU.add)
                    hnview = nx.rearrange("p (a h) -> p a h", a=3)[:, 0::2, :]
                    nc.vector.tensor_tensor(out=tt, in0=rr, in1=hnview,
                                            op=ALU.mult)
                else:
                    l = active[0]
                    nc.scalar.activation(out=tru[:, l], in_=rz[:, l],
                                         func=AF.Tanh, scale=0.5)
                    nc.vector.tensor_scalar(out=rr[:, l], in0=tru[:, l, 0:H],
                                            scalar1=0.5, scalar2=0.5,
                                            op0=ALU.mult, op1=ALU.add)
                    nc.vector.tensor_tensor(out=tt[:, l], in0=rr[:, l],
                                            in1=hn[:, 2 * l], op=ALU.mult)
                if L1:
                    nc.vector.tensor_tensor(out=aa[:, 0], in0=tt[:, 0],
                                            in1=nx[:, H:2 * H], op=ALU.add)
                if L0:
                    nc.vector.tensor_tensor(out=aa[:, 1], in0=tt[:, 1],
                                            in1=c0n, op=ALU.add)

                if len(active) == 2:
                    # u = 0.5+0.5*t_z ; dd = u*h ; v = 1-u = 0.5-0.5*t_z
                    nc.vector.tensor_scalar(out=uu, in0=tru[:, :, H:2 * H],
                                            scalar1=0.5, scalar2=0.5,
                                            op0=ALU.mult, op1=ALU.add)
                    nc.vector.tensor_tensor(out=dd, in0=uu,
                                            in1=hh_prev, op=ALU.mult)
                    nc.vector.tensor_scalar(out=vv, in0=tru[:, :, H:2 * H],
                                            scalar1=-0.5, scalar2=0.5,
                                            op0=ALU.mult, op1=ALU.add)
                    nc.scalar.activation(out=nn.rearrange("p l h -> p (l h)"),
                                         in_=aa.rearrange("p l h -> p (l h)"),
                                         func=AF.Tanh)
                    nc.vector.tensor_tensor(out=mm_, in0=vv, in1=nn, op=ALU.mult)
                    nc.vector.tensor_tensor(out=hh_new, in0=dd, in1=mm_,
                                            op=ALU.add)
                else:
                    l = active[0]
                    nc.vector.tensor_scalar(out=uu[:, l], in0=tru[:, l, H:2 * H],
                                            scalar1=0.5, scalar2=0.5,
                                            op0=ALU.mult, op1=ALU.add)
                    nc.vector.tensor_tensor(out=dd[:, l], in0=uu[:, l],
                                            in1=hh_prev[:, l], op=ALU.mult)
                    nc.vector.tensor_scalar(out=vv[:, l], in0=tru[:, l, H:2 * H],
                                            scalar1=-0.5, scalar2=0.5,
                                            op0=ALU.mult, op1=ALU.add)
                    nc.scalar.activation(out=nn[:, l], in_=aa[:, l], func=AF.Tanh)
                    nc.vector.tensor_tensor(out=mm_[:, l], in0=vv[:, l],
                                            in1=nn[:, l], op=ALU.mult)
                    nc.vector.tensor_tensor(out=hh_new[:, l], in0=dd[:, l],
                                            in1=mm_[:, l], op=ALU.add)
                if s == 0:
                    nc.vector.tensor_copy(out=hh_new[:, 0], in_=hh_prev[:, 0])

                tp = pp.tile([128, 384], F32, tag="tp", bufs=1)

                # defer y-path: copy h1_{s-1} into the pair history buffer
                if L1:
                    i = (s - 1) % 2
                    if i == 0:
                        hist = wp.tile([64, 2, H], F32, tag="hist", bufs=2)
                    nc.scalar.copy(out=hist[:, i, :], in_=hh_new[:, 0])

                # state transposes -> tp[:, 0:256], layout (c, l, b)
                if L0:
                    lset = (0, 1) if L1 else (1,)
                    for c in range(2):
                        for l in lset:
                            nc.tensor.transpose(
                                tp[:, c * 128 + l * 64: c * 128 + (l + 1) * 64],
                                hh_new[:, l, c * 128:(c + 1) * 128],
                                ident[0:64, 0:64])
                    hT_new = wp.tile([128, 2, 2, 64], F32R, tag="hT", bufs=3)
                    if L1:
                        nc.vector.tensor_copy(
                            out=hT_new.rearrange("p c l b -> p (c l b)"),
                            in_=tp[:, 0:256])
                    else:
                        # slot 0: layer-1 half of hT carries the init state
                        nc.vector.tensor_copy(
                            out=hT_new[:, :, 1, :],
                            in_=tp[:, 0:256]
                            .rearrange("p (c b) -> p c b", c=2)[:, :, 64:128])
                        nc.vector.tensor_copy(
                            out=hT_new[:, :, 0, :],
                            in_=hT_prev[:, :, 0, :])
                else:
                    hT_new = hT_prev

                hp_ctx.__exit__(None, None, None)
                if L1 and (s - 1) % 2 == 1:
                    emit_ypair(hist, (s - 1) // 2)

                hh_prev = hh_new
                hT_prev = hT_new

    nc.compile()
    return nc


_cache = {}


def _get_program(flags):
    key = tuple(sorted(flags.items()))
    if key not in _cache:
        _cache[key] = _build(flags)
    return _cache[key]


def kernel(z, W_init, b_init, embedding, W_ih0, W_hh0, b_ih0, b_hh0,
           W_ih1, W_hh1, b_ih1, b_hh1, ln_g, ln_b, W_out, b_out):
    global last_exec_ns, last_results
    z = _np(z); W_init = _np(W_init); b_init = _np(b_init)
    embedding = _np(embedding)
    W_ih0 = _np(W_ih0); W_hh0 = _np(W_hh0); b_ih0 = _np(b_ih0); b_hh0 = _np(b_hh0)
    W_ih1 = _np(W_ih1); W_hh1 = _np(W_hh1); b_ih1 = _np(b_ih1); b_hh1 = _np(b_hh1)
    ln_g = _np(ln_g); ln_b = _np(ln_b); W_out = _np(W_out); b_out = _np(b_out)

    # layer-0 input gates are constant across (b, t): fold embedding @ W_ih0.T
    gx0 = (embedding @ W_ih0.T + b_ih0).reshape(1, 3 * H)
    c0rz = gx0[:, 0:2 * H] + b_hh0[None, 0:2 * H]
    c0n = gx0[:, 2 * H:]
    c1rz = (b_ih1 + b_hh1)[None, 0:2 * H]

    flags = {
        "binit": bool(np.any(b_init != 0)),
        "c1rz": bool(np.any(c1rz != 0)),
        "bhh0n": bool(np.any(b_hh0[2 * H:] != 0)),
        "bhh1n": bool(np.any(b_hh1[2 * H:] != 0)),
        "bih1n": bool(np.any(b_ih1[2 * H:] != 0)),
        "lng": bool(np.any(ln_g != 1.0)),
        "lnb": bool(np.any(ln_b != 0)),
        "bout": bool(np.any(b_out != 0)),
    }
    nc = _get_program(flags)

    common = {
        "winitT": np.ascontiguousarray(W_init.T),
        "whh0T": np.ascontiguousarray(W_hh0.T.reshape(2, 128, 3 * H)),
        "whh1T": np.ascontiguousarray(W_hh1.T.reshape(2, 128, 3 * H)),
        "wih1T": np.ascontiguousarray(W_ih1.T.reshape(2, 128, 3 * H)),
        "woutT": np.ascontiguousarray(W_out.T.reshape(2, 128, P))
            .astype(ml_dtypes.bfloat16),
        "ident": np.eye(128, dtype=np.float32),
        "c0rz": np.ascontiguousarray(c0rz),
        "c0n": np.ascontiguousarray(c0n),
    }
    if flags["binit"]:
        common["binit"] = b_init.reshape(1, H)
    if flags["c1rz"]:
        common["c1rz"] = np.ascontiguousarray(c1rz)
    if flags["bhh0n"]:
        common["bhh0n"] = np.ascontiguousarray(b_hh0[None, 2 * H:])
    if flags["bhh1n"]:
        common["bhh1n"] = np.ascontiguousarray(b_hh1[None, 2 * H:])
    if flags["bih1n"]:
        common["bih1n"] = np.ascontiguousarray(b_ih1[None, 2 * H:])
    if flags["lng"]:
        common["lng"] = ln_g.reshape(1, H)
    if flags["lnb"]:
        common["lnb"] = ln_b.reshape(1, H)
    if flags["bout"]:
        common["bout"] = b_out.reshape(1, P).astype(ml_dtypes.bfloat16)

    in_maps = []
    for c in range(NCORES):
        m = dict(common)
        m["zT"] = np.ascontiguousarray(z[c * BS:(c + 1) * BS].T)
        in_maps.append(m)

    trace = os.environ.get("KERNEL_TRACE", "0") == "1"
    res = run_bass_kernel_spmd(nc, in_maps, core_ids=list(range(NCORES)),
                               trace=trace)
    last_exec_ns = res.exec_time_ns
    last_results = res
    return np.concatenate([r["out"][None] for r in res.results], axis=0) \
             .reshape(B, T, P)


# revision 39
# speedup vs baseline: 1.2445x; 1.0745x over previous
"""Trainium2 Bass kernel for nn_Decoder: 2-layer GRU decoder + LayerNorm + ELU + vocab head.

Contract: kernel(**inputs) takes the FULL unsharded inputs (as produced by the
reference setup_inputs) and returns the FULL (512, 64, 10000) float32 logits.
Internally: data-parallel shard of batch B=512 across 8 NeuronCores; all
weights replicated. Self-contained (shapes hardcoded).

Layout notes (per core, BS=64 batch rows):
- Both GRU layers run interleaved, one slot apart: slot s computes layer-0
  step s and layer-1 step s-1. State tile hh [64, 2, 256] = (b, layer, h)
  with layer 1 in slot 0. GRU matmuls use float32r (full PE rate).
- Output stores go through the gpsimd SWDGE queue: HWDGE (sync/scalar)
  SBUF->DRAM stores serialize onto 2 SDMA engines (~26 GB/s/queue), while
  SWDGE fans descriptors across all 16 engines (~358 GB/s HBM-bound).
  Each store is [128, 5000] f32 = 2.56 MB with 20 KB/partition descriptors.
- Head matmuls in bf16 (yT stationary, W_out.T moving): 2 cols/cycle on the
  PE + half the weight-load traffic. PSUM accumulates f32, logits exact
  enough (~1e-3 rel).
- ACT stays on the exp_and_others table set the whole kernel: tanh for GRU
  gates (sigmoid(x) = 0.5 + 0.5*tanh(x/2)), exp for ELU
  (elu(x) = max(x,0) + exp(min(x,0)) - 1). LN rsqrt = bit-trick + Newton
  (on GpSimd).
"""
import os
import sys

for _p in ("/opt/trn_rl_repo", "/root/.axon_site/_ro/trn_rl_repo"):
    if os.path.isdir(_p) and _p not in sys.path:
        sys.path.append(_p)

import numpy as np
import ml_dtypes
import concourse.bacc as bacc
import concourse.mybir as mybir
import concourse.tile as tile
from concourse.bass_utils import run_bass_kernel_spmd

F32 = mybir.dt.float32
F32R = mybir.dt.float32r
BF16 = mybir.dt.bfloat16
I32 = mybir.dt.int32
AF = mybir.ActivationFunctionType
ALU = mybir.AluOpType

B, Z, H, T, P = 512, 64, 256, 64, 10000
NCORES = 8
BS = B // NCORES
LN_EPS = 1e-5
NCH = 500                 # head N-chunk (PSUM bank = 512 f32; 20*500 = P)
NSTG = 2500               # staging tile cols (vocab quarter, 10KB descs)
RSQRT_NEWTON = 3
G = 8                     # y-path batch (steps)

last_exec_ns = None
last_results = None


def _np(x):
    return np.ascontiguousarray(np.asarray(x, dtype=np.float32))


def _build(flags):
    nc = bacc.Bacc("TRN2", target_bir_lowering=False)

    zT_d = nc.dram_tensor("zT", (Z, BS), F32R, kind="ExternalInput")
    winitT_d = nc.dram_tensor("winitT", (Z, H), F32R, kind="ExternalInput")
    whh0_d = nc.dram_tensor("whh0T", (2, 128, 3 * H), F32R, kind="ExternalInput")
    whh1_d = nc.dram_tensor("whh1T", (2, 128, 3 * H), F32R, kind="ExternalInput")
    wih1_d = nc.dram_tensor("wih1T", (2, 128, 3 * H), F32R, kind="ExternalInput")
    wout_d = nc.dram_tensor("woutT", (2, 128, P), BF16, kind="ExternalInput")
    ident_d = nc.dram_tensor("ident", (128, 128), F32, kind="ExternalInput")
    c0rz_d = nc.dram_tensor("c0rz", (1, 2 * H), F32, kind="ExternalInput")
    c0n_d = nc.dram_tensor("c0n", (1, H), F32, kind="ExternalInput")
    if flags["binit"]:
        binit_d = nc.dram_tensor("binit", (1, H), F32, kind="ExternalInput")
    if flags["c1rz"]:
        c1rz_d = nc.dram_tensor("c1rz", (1, 2 * H), F32R, kind="ExternalInput")
    if flags["bhh0n"]:
        bhh0n_d = nc.dram_tensor("bhh0n", (1, H), F32R, kind="ExternalInput")
    if flags["bhh1n"]:
        bhh1n_d = nc.dram_tensor("bhh1n", (1, H), F32R, kind="ExternalInput")
    if flags["bih1n"]:
        bih1n_d = nc.dram_tensor("bih1n", (1, H), F32R, kind="ExternalInput")
    if flags["lng"]:
        lng_d = nc.dram_tensor("lng", (1, H), F32, kind="ExternalInput")
    if flags["lnb"]:
        lnb_d = nc.dram_tensor("lnb", (1, H), F32, kind="ExternalInput")
    if flags["bout"]:
        bout_d = nc.dram_tensor("bout", (1, P), BF16, kind="ExternalInput")

    out_d = nc.dram_tensor("out", (BS, T, P), F32, kind="ExternalOutput")

    with tile.TileContext(nc) as tc:
        with (
            tc.tile_pool(name="const", bufs=1) as cp,
            tc.tile_pool(name="work", bufs=2) as wp,
            tc.tile_pool(name="psum", bufs=1, space="PSUM") as pp,
        ):
            # ---- constants / weights into SBUF -----------------------------
            zT = cp.tile([Z, BS], F32R)
            winitT = cp.tile([Z, H], F32R)
            whh0 = cp.tile([128, 2, 3 * H], F32R)
            whh1 = cp.tile([128, 2, 3 * H], F32R)
            wih1 = cp.tile([128, 2, 3 * H], F32R)
            wout = cp.tile([128, 2, P], BF16)
            ident = cp.tile([128, 128], F32)
            c0rzb = cp.tile([64, 2 * H], F32)
            nc.sync.dma_start(out=zT, in_=zT_d[:])
            nc.sync.dma_start(out=winitT, in_=winitT_d[:])
            nc.sync.dma_start(out=ident, in_=ident_d[:])
            nc.sync.dma_start(out=c0rzb, in_=c0rz_d[:].partition_broadcast(64))
            nc.sync.dma_start(out=whh0, in_=whh0_d[:].transpose([1, 0, 2]))
            nc.sync.dma_start(out=whh1, in_=whh1_d[:].transpose([1, 0, 2]))
            nc.sync.dma_start(out=wih1, in_=wih1_d[:].transpose([1, 0, 2]))
            nc.sync.dma_start(out=wout, in_=wout_d[:].transpose([1, 0, 2]))

            c0n = cp.tile([64, H], F32)
            nc.sync.dma_start(out=c0n, in_=c0n_d[:].partition_broadcast(64))

            def row_tile(dram, n, w, dt=F32):
                t = cp.tile([n, w], dt)
                if n > 1:
                    nc.sync.dma_start(out=t, in_=dram[:].partition_broadcast(n))
                else:
                    nc.sync.dma_start(out=t, in_=dram[:])
                return t

            binit_t = row_tile(binit_d, 64, H) if flags["binit"] else None
            c1rz_t = row_tile(c1rz_d, 1, 2 * H, F32R) if flags["c1rz"] else None
            bhh0n_t = row_tile(bhh0n_d, 1, H, F32R) if flags["bhh0n"] else None
            bhh1n_t = row_tile(bhh1n_d, 1, H, F32R) if flags["bhh1n"] else None
            bih1n_t = row_tile(bih1n_d, 1, H, F32R) if flags["bih1n"] else None
            lng_t = row_tile(lng_d, 64, H) if flags["lng"] else None
            lnb_t = row_tile(lnb_d, 64, H) if flags["lnb"] else None
            bout_t = row_tile(bout_d, 1, P, BF16) if flags["bout"] else None

            ones_f = cp.tile([1, 128], F32)
            nc.vector.memset(ones_f, 1.0)
            ones128 = cp.tile([1, 128], F32R)
            nc.vector.tensor_copy(out=ones128, in_=ones_f)
            ones = ones128[:, 0:64]
            identb = cp.tile([128, 128], BF16)
            nc.vector.tensor_copy(out=identb, in_=ident)
            onesb = None
            if flags["bout"]:
                onesb = cp.tile([1, 128], BF16)
                nc.vector.tensor_copy(out=onesb, in_=ones_f)

            # ---- helpers ----------------------------------------------------
            def elu(dst, src, width):
                """dst = elu(src) = relu(src) + exp(min(src,0)) - 1."""
                mf = wp.tile([64, H], F32, tag="elu_m", bufs=2)
                pf = wp.tile([64, H], F32, tag="elu_p", bufs=2)
                ef = wp.tile([64, H], F32, tag="elu_e", bufs=2)
                m, p_, e = mf[:, :width], pf[:, :width], ef[:, :width]
                nc.vector.tensor_scalar(out=m, in0=src, scalar1=0.0,
                                        scalar2=None, op0=ALU.min, op1=ALU.bypass)
                nc.scalar.activation(out=p_, in_=src, func=AF.Relu)
                nc.scalar.activation(out=e, in_=m, func=AF.Exp)
                nc.vector.scalar_tensor_tensor(out=dst, in0=e, scalar=-1.0,
                                               in1=p_, op0=ALU.add, op1=ALU.add)

            def emit_head(yT_pair, pair):
                """Head matmuls + staging + SWDGE DMA for timestep pair."""
                yT0 = yT_pair[:, 0].rearrange("p a b -> p (a b)")
                yT1 = yT_pair[:, 1].rearrange("p a b -> p (a b)")
                # Store scheme (microbenched at ~305 GB/s/core): per vocab
                # quarter, [64, 2500] slices with sync always storing t0
                # (partitions 0-63, even SBUF ports) and scalar always t1
                # (odd ports), so the two HWDGE queues drive disjoint port
                # sets concurrently. Quarter-granular staging tiles let each
                # quarter's stores launch as soon as its 5 chunks drain.
                for q in range(4):
                    stg = wp.tile([128, NSTG], F32, tag="stg", bufs=8)
                    for j in range(NSTG // NCH):
                        n = q * (NSTG // NCH) + j
                        hp = pp.tile([128, NCH], F32, tag="head", bufs=3)
                        nc.tensor.matmul(hp, yT0, wout[:, 0, n * NCH:(n + 1) * NCH],
                                         start=True, stop=False)
                        nc.tensor.matmul(hp, yT1, wout[:, 1, n * NCH:(n + 1) * NCH],
                                         start=False, stop=not flags["bout"])
                        if flags["bout"]:
                            nc.tensor.matmul(hp, onesb,
                                             bout_t[:, n * NCH:(n + 1) * NCH],
                                             start=False, stop=True)
                        dst = stg[:, j * NCH:(j + 1) * NCH]
                        if n % 5 < 2:
                            nc.vector.tensor_copy(out=dst, in_=hp)
                        else:
                            nc.scalar.copy(out=dst, in_=hp)
                    v = slice(q * NSTG, (q + 1) * NSTG)
                    nc.sync.dma_start(out=out_d[:, 2 * pair, v],
                                      in_=stg[0:64, :])
                    nc.scalar.dma_start(out=out_d[:, 2 * pair + 1, v],
                                        in_=stg[64:128, :])

            def emit_ypair(hist, pair):
                """Y-path for one timestep pair: LN stats + rsqrt, LN apply +
                ELU, bf16 transposes, head. Emitted every 2 steps so drain
                copies and store DMAs flow continuously."""
                st6 = wp.tile([64, 2, 6], F32, tag="st6")
                mv = wp.tile([64, 2, 2], F32, tag="mv")
                for i in range(2):
                    nc.vector.bn_stats(out=st6[:, i, :], in_=hist[:, i, :])
                    nc.vector.bn_aggr(out=mv[:, i, :], in_=st6[:, i, :])
                ve = wp.tile([64, 2], F32, tag="ve")
                nc.gpsimd.tensor_scalar(out=ve, in0=mv[:, :, 1], scalar1=LN_EPS,
                                        scalar2=None, op0=ALU.add, op1=ALU.bypass)
                yi = wp.tile([64, 2], I32, tag="yi")
                nc.vector.tensor_scalar(out=yi, in0=ve.bitcast(I32), scalar1=1,
                                        scalar2=None, op0=ALU.logical_shift_right,
                                        op1=ALU.bypass)
                nc.vector.tensor_scalar(out=yi, in0=yi, scalar1=-1,
                                        scalar2=0x5F3759DF, op0=ALU.mult,
                                        op1=ALU.add)
                rs = yi.bitcast(F32)
                tn = wp.tile([64, 2], F32, tag="tn")
                for _ in range(RSQRT_NEWTON):
                    nc.gpsimd.tensor_tensor(out=tn, in0=rs, in1=rs, op=ALU.mult)
                    nc.gpsimd.tensor_tensor(out=tn, in0=tn, in1=ve, op=ALU.mult)
                    nc.gpsimd.tensor_scalar(out=tn, in0=tn, scalar1=-0.5,
                                            scalar2=1.5, op0=ALU.mult, op1=ALU.add)
                    nc.gpsimd.tensor_tensor(out=rs, in0=rs, in1=tn, op=ALU.mult)
                # per-step LN apply + ELU, in place on hist
                for i in range(2):
                    nc.vector.tensor_scalar(out=hist[:, i, :], in0=hist[:, i, :],
                                            scalar1=mv[:, i, 0:1],
                                            scalar2=rs[:, i:i + 1],
                                            op0=ALU.subtract, op1=ALU.mult)
                    if flags["lng"]:
                        nc.vector.tensor_tensor(out=hist[:, i, :], in0=hist[:, i, :],
                                                in1=lng_t, op=ALU.mult)
                    if flags["lnb"]:
                        nc.vector.tensor_tensor(out=hist[:, i, :], in0=hist[:, i, :],
                                                in1=lnb_t, op=ALU.add)
                    elu(hist[:, i, :], hist[:, i, :], H)
                # bf16 cast + transpose + head
                histb = wp.tile([64, 2, H], BF16, tag="histb", bufs=2)
                nc.vector.tensor_copy(
                    out=histb.rearrange("p a h -> p (a h)"),
                    in_=hist.rearrange("p a h -> p (a h)"))
                tp_y = pp.tile([128, 384], F32, tag="tp", bufs=1)
                typ = tp_y.bitcast(BF16)[:, 0:256]
                for par in range(2):
                    for c in range(2):
                        nc.tensor.transpose(
                            typ[:, c * 128 + par * 64: c * 128 + (par + 1) * 64],
                            histb[:, par, c * 128:(c + 1) * 128],
                            identb[0:64, 0:64])
                yT_pair = wp.tile([128, 2, 2, 64], BF16, tag="yT", bufs=2)
                nc.vector.tensor_copy(
                    out=yT_pair.rearrange("p c a b -> p (c a b)"),
                    in_=typ[:, 0:256])
                emit_head(yT_pair, pair)

            # ---- init: h0 = elu(z @ W_init.T + b_init) ----------------------
            itp = pp.tile([128, 384], F32, tag="tp", bufs=1)
            nc.tensor.matmul(itp[0:64, 0:H], zT, winitT, start=True, stop=True)
            h0pre = wp.tile([64, H], F32, tag="h0pre")
            if flags["binit"]:
                nc.vector.tensor_tensor(out=h0pre, in0=itp[0:64, 0:H],
                                        in1=binit_t, op=ALU.add)
            else:
                nc.vector.tensor_copy(out=h0pre, in_=itp[0:64, 0:H])
            hh_prev = wp.tile([64, 2, H], F32, tag="hh", bufs=3)
            elu(hh_prev[:, 0, :], h0pre, H)
            nc.vector.tensor_copy(out=hh_prev[:, 1, :], in_=hh_prev[:, 0, :])
            # transpose init state -> hT_prev [128, 2(c), 2(l), 64(b)]
            itp2 = pp.tile([128, 384], F32, tag="tp", bufs=1)
            for l in range(2):
                for c in range(2):
                    nc.tensor.transpose(
                        itp2[:, c * 128 + l * 64: c * 128 + (l + 1) * 64],
                        hh_prev[:, l, c * 128:(c + 1) * 128], ident[0:64, 0:64])
            hT_prev = wp.tile([128, 2, 2, 64], F32R, tag="hT", bufs=3)
            nc.vector.tensor_copy(out=hT_prev.rearrange("p c l b -> p (c l b)"),
                                  in_=itp2[:, 0:256])

            # ---- main loop: slots 0..T --------------------------------------
            for s in range(T + 1):
                L0 = s < T     # layer-0 computes h0_s   (hh slot 1)
                L1 = s >= 1    # layer-1 computes h1_{s-1} (hh slot 0)

                hp_ctx = tc.high_priority()
                hp_ctx.__enter__()
                h1T = lambda k: hT_prev[:, k, 0, :]
                h0T = lambda k: hT_prev[:, k, 1, :]

                # gate matmuls: rz [64, 2, 512] (l1 | l0), nx [64, 768]
                rz = pp.tile([64, 2, 2 * H], F32, tag="rz", bufs=1)
                nx = pp.tile([64, 3 * H], F32, tag="nx", bufs=1)
                if L0:
                    nc.tensor.matmul(rz[:, 1, :], h0T(0), whh0[:, 0, 0:2 * H],
                                     start=True, stop=False)
                    nc.tensor.matmul(rz[:, 1, :], h0T(1), whh0[:, 1, 0:2 * H],
                                     start=False, stop=True)
                    # bias via DVE post-add on PSUM (an N=512 bias matmul
                    # would cost ~0.65us of PE stream per step)
                    nc.vector.tensor_tensor(out=rz[:, 1, :], in0=rz[:, 1, :],
                                            in1=c0rzb, op=ALU.add)
                    nc.tensor.matmul(nx[:, 2 * H:], h0T(0), whh0[:, 0, 2 * H:],
                                     start=True, stop=False)
                    nc.tensor.matmul(nx[:, 2 * H:], h0T(1), whh0[:, 1, 2 * H:],
                                     start=False, stop=not flags["bhh0n"])
                    if flags["bhh0n"]:
                        nc.tensor.matmul(nx[:, 2 * H:], ones, bhh0n_t,
                                         start=False, stop=True)
                if L1:
                    nc.tensor.matmul(rz[:, 0, :], h1T(0), whh1[:, 0, 0:2 * H],
                                     start=True, stop=False)
                    nc.tensor.matmul(rz[:, 0, :], h1T(1), whh1[:, 1, 0:2 * H],
                                     start=False, stop=False)
                    nc.tensor.matmul(rz[:, 0, :], h0T(0), wih1[:, 0, 0:2 * H],
                                     start=False, stop=False)
                    nc.tensor.matmul(rz[:, 0, :], h0T(1), wih1[:, 1, 0:2 * H],
                                     start=False, stop=not flags["c1rz"])
                    if flags["c1rz"]:
                        nc.tensor.matmul(rz[:, 0, :], ones, c1rz_t,
                                         start=False, stop=True)
                    nc.tensor.matmul(nx[:, 0:H], h1T(0), whh1[:, 0, 2 * H:],
                                     start=True, stop=False)
                    nc.tensor.matmul(nx[:, 0:H], h1T(1), whh1[:, 1, 2 * H:],
                                     start=False, stop=not flags["bhh1n"])
                    if flags["bhh1n"]:
                        nc.tensor.matmul(nx[:, 0:H], ones, bhh1n_t,
                                         start=False, stop=True)
                    nc.tensor.matmul(nx[:, H:2 * H], h0T(0), wih1[:, 0, 2 * H:],
                                     start=True, stop=False)
                    nc.tensor.matmul(nx[:, H:2 * H], h0T(1), wih1[:, 1, 2 * H:],
                                     start=False, stop=not flags["bih1n"])
                    if flags["bih1n"]:
                        nc.tensor.matmul(nx[:, H:2 * H], ones, bih1n_t,
                                         start=False, stop=True)

                # gates via tanh: sigmoid(x) = 0.5 + 0.5*tanh(x/2).
                # W_hh n-gate columns are prescaled 0.5 on the host, so
                # r*hn = (t_r + 1) * hn' in one STT; likewise
                # u*h = ((t_z + 1) * h) * 0.5 folded into the final STT.
                tru = wp.tile([64, 2, 2 * H], F32, tag="ru")
                tt = wp.tile([64, 2, H], F32, tag="tt")
                aa = wp.tile([64, 2, H], F32, tag="aa")
                nn = wp.tile([64, 2, H], F32, tag="nn")
                dd = wp.tile([64, 2, H], F32, tag="tt")
                vv = wp.tile([64, 2, H], F32, tag="vv")
                mm_ = wp.tile([64, 2, H], F32, tag="aa")
                hh_new = wp.tile([64, 2, H], F32, tag="hh", bufs=3)
                # hn columns per layer: l1 -> nx[:, 0:H], l0 -> nx[:, 2H:3H]
                hn = nx.rearrange("p (a h) -> p a h", a=3)

                active = ([0] if L1 else []) + ([1] if L0 else [])
                if len(active) == 2:
                    nc.scalar.activation(out=tru.rearrange("p l w -> p (l w)"),
                                         in_=rz.rearrange("p l w -> p (l w)"),
                                         func=AF.Tanh, scale=0.5)
                    hnview = nx.rearrange("p (a h) -> p a h", a=3)[:, 0::2, :]
                    nc.vector.scalar_tensor_tensor(out=tt, in0=tru[:, :, 0:H],
                                                   scalar=1.0, in1=hnview,
                                                   op0=ALU.add, op1=ALU.mult)
                else:
                    l = active[0]
                    nc.scalar.activation(out=tru[:, l], in_=rz[:, l],
                                         func=AF.Tanh, scale=0.5)
                    nc.vector.scalar_tensor_tensor(out=tt[:, l],
                                                   in0=tru[:, l, 0:H],
                                                   scalar=1.0, in1=hn[:, 2 * l],
                                                   op0=ALU.add, op1=ALU.mult)
                if L1:
                    nc.vector.tensor_tensor(out=aa[:, 0], in0=tt[:, 0],
                                            in1=nx[:, H:2 * H], op=ALU.add)
                if L0:
                    nc.vector.tensor_tensor(out=aa[:, 1], in0=tt[:, 1],
                                            in1=c0n, op=ALU.add)

                if len(active) == 2:
                    nc.vector.scalar_tensor_tensor(out=dd, in0=tru[:, :, H:2 * H],
                                                   scalar=1.0, in1=hh_prev,
                                                   op0=ALU.add, op1=ALU.mult)
                    nc.vector.tensor_scalar(out=vv, in0=tru[:, :, H:2 * H],
                                            scalar1=-0.5, scalar2=0.5,
                                            op0=ALU.mult, op1=ALU.add)
                    nc.scalar.activation(out=nn.rearrange("p l h -> p (l h)"),
                                         in_=aa.rearrange("p l h -> p (l h)"),
                                         func=AF.Tanh)
                    nc.vector.tensor_tensor(out=mm_, in0=vv, in1=nn, op=ALU.mult)
                    nc.vector.scalar_tensor_tensor(out=hh_new, in0=dd,
                                                   scalar=0.5, in1=mm_,
                                                   op0=ALU.mult, op1=ALU.add)
                else:
                    l = active[0]
                    nc.vector.scalar_tensor_tensor(out=dd[:, l],
                                                   in0=tru[:, l, H:2 * H],
                                                   scalar=1.0, in1=hh_prev[:, l],
                                                   op0=ALU.add, op1=ALU.mult)
                    nc.vector.tensor_scalar(out=vv[:, l], in0=tru[:, l, H:2 * H],
                                            scalar1=-0.5, scalar2=0.5,
                                            op0=ALU.mult, op1=ALU.add)
                    nc.scalar.activation(out=nn[:, l], in_=aa[:, l], func=AF.Tanh)
                    nc.vector.tensor_tensor(out=mm_[:, l], in0=vv[:, l],
                                            in1=nn[:, l], op=ALU.mult)
                    nc.vector.scalar_tensor_tensor(out=hh_new[:, l], in0=dd[:, l],
                                                   scalar=0.5, in1=mm_[:, l],
                                                   op0=ALU.mult, op1=ALU.add)
                if s == 0:
                    nc.vector.tensor_copy(out=hh_new[:, 0], in_=hh_prev[:, 0])

                tp = pp.tile([128, 384], F32, tag="tp", bufs=1)

                # defer y-path: copy h1_{s-1} into the pair history buffer
                if L1:
                    i = (s - 1) % 2
                    if i == 0:
                        hist = wp.tile([64, 2, H], F32, tag="hist", bufs=2)
                    nc.scalar.copy(out=hist[:, i, :], in_=hh_new[:, 0])

                # state transposes -> tp[:, 0:256], layout (c, l, b)
                if L0:
                    lset = (0, 1) if L1 else (1,)
                    for c in range(2):
                        for l in lset:
                            nc.tensor.transpose(
                                tp[:, c * 128 + l * 64: c * 128 + (l + 1) * 64],
                                hh_new[:, l, c * 128:(c + 1) * 128],
                                ident[0:64, 0:64])
                    hT_new = wp.tile([128, 2, 2, 64], F32R, tag="hT", bufs=3)
                    if L1:
                        nc.vector.tensor_copy(
                            out=hT_new.rearrange("p c l b -> p (c l b)"),
                            in_=tp[:, 0:256])
                    else:
                        # slot 0: layer-1 half of hT carries the init state
                        nc.vector.tensor_copy(
                            out=hT_new[:, :, 1, :],
                            in_=tp[:, 0:256]
                            .rearrange("p (c b) -> p c b", c=2)[:, :, 64:128])
                        nc.vector.tensor_copy(
                            out=hT_new[:, :, 0, :],
                            in_=hT_prev[:, :, 0, :])
                else:
                    hT_new = hT_prev

                hp_ctx.__exit__(None, None, None)
                if L1 and (s - 1) % 2 == 1:
                    emit_ypair(hist, (s - 1) // 2)

                hh_prev = hh_new
                hT_prev = hT_new

    nc.compile()
    return nc


_cache = {}


def _get_program(flags):
    key = tuple(sorted(flags.items()))
    if key not in _cache:
        _cache[key] = _build(flags)
    return _cache[key]


def kernel(z, W_init, b_init, embedding, W_ih0, W_hh0, b_ih0, b_hh0,
           W_ih1, W_hh1, b_ih1, b_hh1, ln_g, ln_b, W_out, b_out):
    global last_exec_ns, last_results
    z = _np(z); W_init = _np(W_init); b_init = _np(b_init)
    embedding = _np(embedding)
    W_ih0 = _np(W_ih0); W_hh0 = _np(W_hh0); b_ih0 = _np(b_ih0); b_hh0 = _np(b_hh0)
    W_ih1 = _np(W_ih1); W_hh1 = _np(W_hh1); b_ih1 = _np(b_ih1); b_hh1 = _np(b_hh1)
    ln_g = _np(ln_g); ln_b = _np(ln_b); W_out = _np(W_out); b_out = _np(b_out)

    # layer-0 input gates are constant across (b, t): fold embedding @ W_ih0.T
    gx0 = (embedding @ W_ih0.T + b_ih0).reshape(1, 3 * H)
    c0rz = gx0[:, 0:2 * H] + b_hh0[None, 0:2 * H]
    c0n = gx0[:, 2 * H:]
    c1rz = (b_ih1 + b_hh1)[None, 0:2 * H]

    flags = {
        "binit": bool(np.any(b_init != 0)),
        "c1rz": bool(np.any(c1rz != 0)),
        "bhh0n": bool(np.any(b_hh0[2 * H:] != 0)),
        "bhh1n": bool(np.any(b_hh1[2 * H:] != 0)),
        "bih1n": bool(np.any(b_ih1[2 * H:] != 0)),
        "lng": bool(np.any(ln_g != 1.0)),
        "lnb": bool(np.any(ln_b != 0)),
        "bout": bool(np.any(b_out != 0)),
    }
    nc = _get_program(flags)

    # prescale recurrent n-gate columns by 0.5: the kernel computes
    # r*hn as (tanh(rz/2) + 1) * (0.5*hn)
    whh0s = W_hh0.T.reshape(2, 128, 3 * H).copy()
    whh0s[:, :, 2 * H:] *= 0.5
    whh1s = W_hh1.T.reshape(2, 128, 3 * H).copy()
    whh1s[:, :, 2 * H:] *= 0.5
    common = {
        "winitT": np.ascontiguousarray(W_init.T),
        "whh0T": np.ascontiguousarray(whh0s),
        "whh1T": np.ascontiguousarray(whh1s),
        "wih1T": np.ascontiguousarray(W_ih1.T.reshape(2, 128, 3 * H)),
        "woutT": np.ascontiguousarray(W_out.T.reshape(2, 128, P))
            .astype(ml_dtypes.bfloat16),
        "ident": np.eye(128, dtype=np.float32),
        "c0rz": np.ascontiguousarray(c0rz),
        "c0n": np.ascontiguousarray(c0n),
    }
    if flags["binit"]:
        common["binit"] = b_init.reshape(1, H)
    if flags["c1rz"]:
        common["c1rz"] = np.ascontiguousarray(c1rz)
    if flags["bhh0n"]:
        common["bhh0n"] = np.ascontiguousarray(0.5 * b_hh0[None, 2 * H:])
    if flags["bhh1n"]:
        common["bhh1n"] = np.ascontiguousarray(0.5 * b_hh1[None, 2 * H:])
    if flags["bih1n"]:
        common["bih1n"] = np.ascontiguousarray(b_ih1[None, 2 * H:])
    if flags["lng"]:
        common["lng"] = ln_g.reshape(1, H)
    if flags["lnb"]:
        common["lnb"] = ln_b.reshape(1, H)
    if flags["bout"]:
        common["bout"] = b_out.reshape(1, P).astype(ml_dtypes.bfloat16)

    in_maps = []
    for c in range(NCORES):
        m = dict(common)
        m["zT"] = np.ascontiguousarray(z[c * BS:(c + 1) * BS].T)
        in_maps.append(m)

    trace = os.environ.get("KERNEL_TRACE", "0") == "1"
    res = run_bass_kernel_spmd(nc, in_maps, core_ids=list(range(NCORES)),
                               trace=trace)
    last_exec_ns = res.exec_time_ns
    last_results = res
    return np.concatenate([r["out"][None] for r in res.results], axis=0) \
             .reshape(B, T, P)
